# revision 40
# baseline (speedup 1.0000x reference)
"""Trainium2 Bass kernel: per-(head,batch) euclidean compatibility matrix,
globally min/max-rescaled to [-9, 9].

reference (jax):
    q_sq = sum(Q*Q, -1)[..., :, None]
    k_sq = sum(K*K, -1)[..., None, :]
    cross = einsum("hbqd,hbgd->hbqg", Q, K)
    compat = sqrt(q_sq + k_sq - 2*cross)
    out = A_LO + (compat - min) * (A_HI - A_LO) / (max - min)   # min/max per (h,b)

Sharding: head h -> NeuronCore h (8 heads, 8 cores), fully independent.

Per-core program (B=4 slices of [N=2048, D=16]):
  phase A (slice 0 up front, rest emitted mid-previous-slice): load Q/K
    natural layout [128, 16, 18] fp32, DVE row sum-of-squares; K gets
    (k_hi, k_lo) fp16-split sum-of-squares columns, Q gets const -0.5
    columns and a per-partition q_sq tile (fp32, fed to ACT as bias).
    PE-transposes (fp32; fp16 PSUM writes crash TRN2 - PSUM is fp32-only
    here) build fp16 UT = [Q^T; -.5; -.5], VT = [K^T; k_hi; k_lo]:
      psum = UT[:,q]^T @ VT[:,g] = QK - 0.5*k_sq
      d2   = -2*psum + q_sq      (ACT pre-affine, scale=-2)
  phase B per supertile (4 q-tiles, [128, 8192] fp16): 16 fp16 matmuls
    -> PSUM, ACT sqrt(scale*x+bias) PSUM->fp16 SBUF, 2 fp16 DVE fold
    trees (2x mode) + fused accum (min via negate / max partials).
    NOTE: nc.vector.tensor_tensor_reduce would fuse each tree into one
    instruction but CRASHES TRN2 hardware (NRT_EXEC_UNIT_UNRECOVERABLE,
    verified by micro-test even in the f32/add form qr.py uses).
    GPSIMD can't help either: TENSOR_TENSOR fails walrus' Pool-engine
    ISA check, and gpsimd tensor_reduce is partition-axis only.
  phase C: finalize min/max across supertiles + partitions (gpsimd
    all-reduce), c1 = 18/(max-min), c0 = -9 + (-min)*c1.
  phase D, distributed piecewise one slice behind: per supertile, affine
    (DVE fp16 4x; ACT_AFFINES of them on ACT to rebalance the ~204 vs
    ~149us DVE/ACT load) + DMA to DRAM fp16.  Pieces are emitted AFTER
    the next slice's same-index fold block so the in-order ACT queue
    never parks an affine between sqrt drains and the folds waiting on
    them; the last piece fills the gpsimd-allreduce gap; the last
    slice's affines all stay on DVE (its tail is DMA-bound).

Engine budget per core (measured): DVE ~193us (fold trees 154 + affines
+ phase A), ACT ~184us (64 sqrt drains + 8 UV copies + 6 affines), PE
~141us, DMA ~114us; span ~250us = DVE + warmup ~17 + DMA tail ~24.

Output is fp16 on device (halves HBM write traffic); host upcasts to
fp32.  rel-err budget 2e-2 >> fp16 quantization (~1e-3)."""

import numpy as np

H, B, N, D = 8, 4, 2048, 16
A_LO, A_HI = -9.0, 9.0
P = 128
R = D + 2            # matmul contraction rows: 16 data + k_hi + k_lo
NT = N // P          # 16 q-tiles per slice
STW = 4              # q-tiles per supertile
ST = NT // STW       # 4 supertiles per slice

# ---- tuning knobs ----
SQ_BUFS = 8          # SBUF supertile bufs of [128, 8192] fp16
ACT_COPIES = 8       # of the 8 phase-A PSUM->SBUF copies routed to ACT (rest DVE)
ACT_AFFINES = 6      # of the 12 non-last-slice affines routed to ACT (rebalance:
                     # DVE bottleneck ~204us vs ACT ~149us; each move costs
                     # ACT 7.2us to save DVE 2.5us; x=6 lands ~193/~184)
EXTRA_ACT_AFF = 5    # odd supertile index given a 7th ACT affine (-1 = off);
                     # the even-index alternation above caps at 6
MM_N = 512           # matmul moving free dim (1 PSUM bank; 1024 fails ISA check)

_CACHE = {}


def build_program():
    import concourse.bacc as bacc
    import concourse.bass as bass
    import concourse.mybir as mybir
    from concourse import tile, masks
    from concourse import bass_isa

    f32 = mybir.dt.float32
    f16 = mybir.dt.float16
    Alu = mybir.AluOpType
    AF = mybir.ActivationFunctionType
    AX = mybir.AxisListType

    nc = bacc.Bacc()
    Qd = nc.declare_dram_parameter("Q", [B, N, D], f32, isOutput=False)
    Kd = nc.declare_dram_parameter("K", [B, N, D], f32, isOutput=False)
    Od = nc.declare_dram_parameter("out", [B, N, N], f16, isOutput=True)

    with tile.TileContext(nc) as tc:
        with (
            tc.tile_pool(name="const", bufs=1) as constp,
            tc.tile_pool(name="ld", bufs=8) as ldp,
            tc.tile_pool(name="sqt", bufs=2) as sqtp,
            tc.tile_pool(name="qsq", bufs=4) as qsqp,
            tc.tile_pool(name="ksq", bufs=2) as ksqp,
            tc.tile_pool(name="uv", bufs=8) as uvp,
            tc.tile_pool(name="sq", bufs=SQ_BUFS) as sqp,
            tc.tile_pool(name="scr", bufs=1) as scrp,
            tc.tile_pool(name="small", bufs=2) as smallp,
            tc.tile_pool(name="ps", bufs=2, space=bass.MemorySpace.PSUM) as psp,
        ):
            ident = constp.tile([P, P], f32)
            masks.make_identity(nc, ident[:])

            # PE p-state pre-warm: the PE DVFS ramps (0.65 -> 1.2 -> 2.4 GHz)
            # only while continuously busy; dummy transposes during the input
            # DMA wait start the ramp so slice 0's real transposes/matmuls
            # don't run at the cold clock.
            warm = psp.tile([P, N], f32, tag="ps")
            for t in range(NT):
                nc.tensor.transpose(
                    warm[:, t * P : (t + 1) * P], ident[:], ident[:]
                )

            # ---------------- phase A: build UT / VT / qsq for all slices ----
            # prefetch: all 8 input DMAs in flight up front
            lds = []
            for b in range(B):
                for (src, is_k) in ((Qd, False), (Kd, True)):
                    ld = ldp.tile([P, NT, R], f32, tag="ld")
                    if not is_k:
                        nc.gpsimd.memset(ld[:], -0.5)
                    nc.sync.dma_start(
                        ld[:, :, 0:D], src[b].rearrange("(t p) d -> p t d", p=P)
                    )
                    lds.append(ld)
            UTs, VTs, qsqs = [], [], []
            copy_idx = [0]

            def emit_phase_a(b):
                for (src, is_k) in ((Qd, False), (Kd, True)):
                    ld = lds[b * 2 + (1 if is_k else 0)]
                    sqt = sqtp.tile([P, NT, D], f32, tag="sqt")
                    nc.vector.tensor_tensor(
                        sqt[:], ld[:, :, 0:D], ld[:, :, 0:D], Alu.mult
                    )
                    if is_k:
                        ksq = ksqp.tile([P, NT], f32, tag="ksq")
                        nc.vector.tensor_reduce(ksq[:], sqt[:], AX.X, Alu.add)
                        khil = ksqp.tile([P, NT], f16, tag="khil")
                        nc.vector.tensor_copy(khil[:], ksq[:])
                        nc.vector.tensor_copy(ld[:, :, D], khil[:])
                        nc.vector.tensor_tensor(
                            ld[:, :, D + 1], ksq[:], ld[:, :, D], Alu.subtract
                        )
                    else:
                        qsq = qsqp.tile([P, NT], f32, tag="qsq")
                        nc.vector.tensor_reduce(qsq[:], sqt[:], AX.X, Alu.add)
                        qsqs.append(qsq)
                    tp = psp.tile([R, N], f32, tag="ps")
                    for t in range(NT):
                        nc.tensor.transpose(
                            tp[:, t * P : (t + 1) * P], ld[:, t, :], ident[:]
                        )
                    TT = uvp.tile([R, N], f16, tag="uv")
                    if copy_idx[0] < ACT_COPIES:
                        nc.scalar.copy(TT[:], tp[:])
                    else:
                        nc.vector.tensor_copy(TT[:], tp[:])
                    copy_idx[0] += 1
                    (VTs if is_k else UTs).append(TT)

            emit_phase_a(0)

            # ---------------- phases B-D per slice ----------------
            def emit_phase_d_piece(b, sts, c0, c1, s, last=False):
                # One supertile's affine + store.  Distributed piecewise into
                # the NEXT slice's supertile loop so the in-order ACT/DVE
                # queues interleave affines between that slice's sqrt drains
                # (instead of parking a 4-burst behind them), the DMA stream
                # stays smooth, and sq bufs recycle early.
                # last slice: ALL affines on DVE -- the tail is DMA-bound and
                # the ACT queue lags the DVE queue there, so an ACT affine
                # would only delay the first tail DMA.  Earlier slices:
                # alternate ACT_AFFINES of the 12 onto ACT (ACT has ~55us of
                # slack vs the DVE bottleneck; 7.2us there buys 2.5us here).
                st = sts[s]
                g = b * ST + s
                if not last and (
                    (g % 2 == 0 and g // 2 < ACT_AFFINES) or g == EXTRA_ACT_AFF
                ):
                    nc.scalar.activation(
                        st[:],
                        st[:],
                        AF.Identity,
                        bias=c0[:, 0:1],
                        scale=c1[:, 0:1],
                    )
                elif last:
                    # tail pieces: affine+store in HALF-supertile chunks so
                    # the DMA-bound tail stream starts ~1.2us earlier
                    half = STW * N // 2
                    for h in range(2):
                        nc.vector.tensor_scalar(
                            st[:, h * half : (h + 1) * half],
                            st[:, h * half : (h + 1) * half],
                            c1[:, 0:1], c0[:, 0:1], Alu.mult, Alu.add,
                        )
                        nc.sync.dma_start(
                            Od[
                                b,
                                (s * STW + h * 2) * P : (s * STW + (h + 1) * 2) * P,
                                :,
                            ].rearrange("(t p) n -> p t n", p=P),
                            st[:, h * half : (h + 1) * half].rearrange(
                                "p (t n) -> p t n", n=N
                            ),
                        )
                    return
                else:
                    nc.vector.tensor_scalar(
                        st[:], st[:], c1[:, 0:1], c0[:, 0:1], Alu.mult, Alu.add
                    )
                nc.sync.dma_start(
                    Od[b, s * STW * P : (s + 1) * STW * P, :].rearrange(
                        "(t p) n -> p t n", p=P
                    ),
                    st[:].rearrange("p (t n) -> p t n", n=N),
                )

            pend = None
            for b in range(B):
                UT, VT, qsq = UTs[b], VTs[b], qsqs[b]
                minp = smallp.tile([P, ST], f32, tag="minp")
                maxp = smallp.tile([P, ST], f32, tag="maxp")
                sts = []
                for s in range(ST):
                    if s == 1 and b + 1 < B:
                        emit_phase_a(b + 1)
                    st = sqp.tile([P, STW * N], f16, tag="sq")
                    # min/max via fp16 TT fold trees (2x mode; accum ops are
                    # stuck at 1x for 16-bit so fold first, accum on 512).
                    # The first fold level is SPLIT into two q-tile pairs so
                    # folding starts after TWO sqrt drains instead of four --
                    # at slice starts and warmup the DVE otherwise idles
                    # ~4-6us waiting for the full supertile.
                    # minp holds NEGATED minima (max of -x at the accum).
                    tas = {}
                    for t in range(STW):
                        i = s * STW + t
                        d2 = psp.tile([P, N], f32, tag="ps")
                        lhs = UT[:, i * P : (i + 1) * P]
                        for j in range(N // MM_N):
                            nc.tensor.matmul(
                                d2[:, j * MM_N : (j + 1) * MM_N],
                                lhs,
                                VT[:, j * MM_N : (j + 1) * MM_N],
                                start=True,
                                stop=True,
                            )
                        nc.scalar.activation(
                            st[:, t * N : (t + 1) * N],
                            d2[:],
                            AF.Sqrt,
                            bias=qsq[:, i : i + 1],
                            scale=-2.0,
                        )
                        if t == 1:
                            for (alu, sub) in ((Alu.max, "M"), (Alu.min, "m")):
                                ta = scrp.tile([P, 2048], f16, tag=f"ta{sub}")
                                nc.vector.tensor_tensor(
                                    ta[:], st[:, 0:2048], st[:, 2048:4096], alu
                                )
                                tas[sub] = ta
                    for (alu, sgn, sub, partial) in (
                        (Alu.max, 1.0, "M", maxp),
                        (Alu.min, -1.0, "m", minp),
                    ):
                        ta = tas[sub]
                        tb = scrp.tile([P, 2048], f16, tag=f"tb{sub}")
                        nc.vector.tensor_tensor(
                            tb[:], st[:, 4096:6144], st[:, 6144:8192], alu
                        )
                        # combine pairs in place: ta = alu(ta, tb)
                        nc.vector.tensor_tensor(ta[:], ta[:], tb[:], alu)
                        t3 = scrp.tile([P, 1024], f16, tag="t3")
                        nc.vector.tensor_tensor(
                            t3[:], ta[:, 0:1024], ta[:, 1024:2048], alu
                        )
                        t4 = scrp.tile([P, 512], f16, tag="t4")
                        nc.vector.tensor_tensor(
                            t4[:], t3[:, 0:512], t3[:, 512:1024], alu
                        )
                        t5 = scrp.tile([P, 512], f16, tag="t5")
                        nc.vector.tensor_scalar(
                            t5[:],
                            t4[:],
                            sgn,
                            None,
                            Alu.mult,
                            Alu.max,
                            accum_out=partial[:, s : s + 1],
                        )
                    sts.append(st)
                    # prev slice's supertile-s affine+store, emitted AFTER
                    # this supertile's folds: the in-order ACT queue then
                    # never parks an affine between a supertile's sqrt
                    # drains and the DVE folds waiting on them
                    if pend is not None and s < ST - 1:
                        emit_phase_d_piece(*pend, s)

                # ---------------- phase C: finalize scalars ----------------
                s2 = smallp.tile([P, 2], f32, tag="s2")
                sr = smallp.tile([P, 2], f32, tag="sr")
                u = smallp.tile([P, 1], f32, tag="u")
                r = smallp.tile([P, 1], f32, tag="r")
                c1 = smallp.tile([P, 1], f32, tag="c1")
                t0 = smallp.tile([P, 1], f32, tag="t0")
                c0 = smallp.tile([P, 1], f32, tag="c0")

                nc.vector.tensor_reduce(s2[:, 0:1], minp[:], AX.X, Alu.max)
                nc.vector.tensor_reduce(s2[:, 1:2], maxp[:], AX.X, Alu.max)
                nc.gpsimd.partition_all_reduce(
                    sr[:], s2[:], P, bass_isa.ReduceOp.max
                )
                # prev slice's final supertile executes on DVE/ACT while
                # gpsimd runs the partition reduce for this slice
                if pend is not None:
                    emit_phase_d_piece(*pend, ST - 1)
                    pend = None
                nmn = sr[:, 0:1]
                mx = sr[:, 1:2]
                nc.vector.tensor_tensor(u[:], mx, nmn, Alu.add)
                nc.vector.reciprocal(r[:], u[:])
                nc.vector.tensor_scalar(c1[:], r[:], A_HI - A_LO, None, Alu.mult)
                nc.vector.tensor_tensor(t0[:], nmn, c1[:], Alu.mult)
                nc.vector.tensor_scalar(c0[:], t0[:], A_LO, None, Alu.add)

                pend = (b, sts, c0, c1)
            for s in range(ST):
                emit_phase_d_piece(*pend, s, last=True)

    nc.compile()
    return nc


def get_program():
    if "nc" not in _CACHE:
        _CACHE["nc"] = build_program()
    return _CACHE["nc"]


def run(inputs, trace=False):
    Q = np.ascontiguousarray(np.asarray(inputs["Q"], dtype=np.float32))
    K = np.ascontiguousarray(np.asarray(inputs["K"], dtype=np.float32))
    assert Q.shape == (H, B, N, D) and K.shape == (H, B, N, D)

    from concourse.bass_utils import run_bass_kernel_spmd

    nc = get_program()
    in_maps = [{"Q": Q[h], "K": K[h]} for h in range(H)]
    res = run_bass_kernel_spmd(nc, in_maps, core_ids=list(range(H)), trace=trace)
    out = np.stack(
        [np.asarray(res.results[h]["out"]) for h in range(H)], axis=0
    ).astype(np.float32)
    return out, res


def kernel(**inputs) -> np.ndarray:
    out, _ = run(inputs, trace=False)
    return out


if __name__ == "__main__":
    nc = get_program()
    print("build ok:", nc)


# revision 41
# speedup vs baseline: 1.0427x; 1.0427x over previous
"""Trainium2 Bass kernel: per-(head,batch) euclidean compatibility matrix,
globally min/max-rescaled to [-9, 9].

reference (jax):
    q_sq = sum(Q*Q, -1)[..., :, None]
    k_sq = sum(K*K, -1)[..., None, :]
    cross = einsum("hbqd,hbgd->hbqg", Q, K)
    compat = sqrt(q_sq + k_sq - 2*cross)
    out = A_LO + (compat - min) * (A_HI - A_LO) / (max - min)   # min/max per (h,b)

Sharding: head h -> NeuronCore h (8 heads, 8 cores), fully independent.

Per-core program (B=4 slices of [N=2048, D=16]):
  phase A (slice 0 up front, rest emitted mid-previous-slice): load Q/K
    natural layout [128, 16, 18] fp32, DVE row sum-of-squares; K gets
    (k_hi, k_lo) fp16-split sum-of-squares columns, Q gets const -0.5
    columns and a per-partition q_sq tile (fp32, fed to ACT as bias).
    PE-transposes (fp32; fp16 PSUM writes crash TRN2 - PSUM is fp32-only
    here) build fp16 UT = [Q^T; -.5; -.5], VT = [K^T; k_hi; k_lo]:
      psum = UT[:,q]^T @ VT[:,g] = QK - 0.5*k_sq
      d2   = -2*psum + q_sq      (ACT pre-affine, scale=-2)
  phase B per supertile (4 q-tiles, [128, 8192] fp16): 16 fp16 matmuls
    -> PSUM, ACT sqrt(scale*x+bias) PSUM->fp16 SBUF, 2 fp16 DVE fold
    trees (2x mode) + fused accum (min via negate / max partials).
    NOTE: nc.vector.tensor_tensor_reduce would fuse each tree into one
    instruction but CRASHES TRN2 hardware (NRT_EXEC_UNIT_UNRECOVERABLE,
    verified by micro-test even in the f32/add form qr.py uses).
    GPSIMD can't help either: TENSOR_TENSOR fails walrus' Pool-engine
    ISA check, and gpsimd tensor_reduce is partition-axis only.
  phase C: finalize min/max across supertiles + partitions (gpsimd
    all-reduce), c1 = 18/(max-min), c0 = -9 + (-min)*c1.
  phase D, distributed piecewise one slice behind: per supertile, affine
    (DVE fp16 4x; ACT_AFFINES of them on ACT to rebalance the ~204 vs
    ~149us DVE/ACT load) + DMA to DRAM fp16.  Pieces are emitted AFTER
    the next slice's same-index fold block so the in-order ACT queue
    never parks an affine between sqrt drains and the folds waiting on
    them; the last piece fills the gpsimd-allreduce gap; the last
    slice's affines all stay on DVE (its tail is DMA-bound).

Engine budget per core (measured): DVE ~193us (fold trees 154 + affines
+ phase A), ACT ~184us (64 sqrt drains + 8 UV copies + 6 affines), PE
~141us, DMA ~114us; span ~250us = DVE + warmup ~17 + DMA tail ~24.

Output is fp16 on device (halves HBM write traffic); host upcasts to
fp32.  rel-err budget 2e-2 >> fp16 quantization (~1e-3)."""

import numpy as np

H, B, N, D = 8, 4, 2048, 16
A_LO, A_HI = -9.0, 9.0
P = 128
R = D + 2            # matmul contraction rows: 16 data + k_hi + k_lo
NT = N // P          # 16 q-tiles per slice
STW = 4              # q-tiles per supertile
ST = NT // STW       # 4 supertiles per slice

# ---- tuning knobs ----
SQ_BUFS = 8          # SBUF supertile bufs of [128, 8192] fp16
ACT_COPIES = 8       # of the 8 phase-A PSUM->SBUF copies routed to ACT (rest DVE)
ACT_AFFINES = 6      # of the 12 non-last-slice affines routed to ACT (rebalance:
                     # DVE bottleneck ~204us vs ACT ~149us; each move costs
                     # ACT 7.2us to save DVE 2.5us; x=6 lands ~193/~184)
EXTRA_ACT_AFF = -1   # odd supertile index given a 7th ACT affine (-1 = off);
                     # the even-index alternation above caps at 6
MM_N = 512           # matmul moving free dim (1 PSUM bank; 1024 fails ISA check)

_CACHE = {}


def build_program():
    import concourse.bacc as bacc
    import concourse.bass as bass
    import concourse.mybir as mybir
    from concourse import tile, masks
    from concourse import bass_isa

    f32 = mybir.dt.float32
    f16 = mybir.dt.float16
    Alu = mybir.AluOpType
    AF = mybir.ActivationFunctionType
    AX = mybir.AxisListType

    nc = bacc.Bacc()
    Qd = nc.declare_dram_parameter("Q", [B, N, D], f32, isOutput=False)
    Kd = nc.declare_dram_parameter("K", [B, N, D], f32, isOutput=False)
    Od = nc.declare_dram_parameter("out", [B, N, N], f16, isOutput=True)

    with tile.TileContext(nc) as tc:
        with (
            tc.tile_pool(name="const", bufs=1) as constp,
            tc.tile_pool(name="ld", bufs=8) as ldp,
            tc.tile_pool(name="sqt", bufs=2) as sqtp,
            tc.tile_pool(name="qsq", bufs=4) as qsqp,
            tc.tile_pool(name="ksq", bufs=2) as ksqp,
            tc.tile_pool(name="uv", bufs=8) as uvp,
            tc.tile_pool(name="sq", bufs=SQ_BUFS) as sqp,
            tc.tile_pool(name="scr", bufs=1) as scrp,
            tc.tile_pool(name="small", bufs=2) as smallp,
            tc.tile_pool(name="ps", bufs=2, space=bass.MemorySpace.PSUM) as psp,
        ):
            ident = constp.tile([P, P], f32)
            masks.make_identity(nc, ident[:])

            # PE p-state pre-warm: the PE DVFS ramps (0.65 -> 1.2 -> 2.4 GHz)
            # only while continuously busy; dummy transposes during the input
            # DMA wait start the ramp so slice 0's real transposes/matmuls
            # don't run at the cold clock.
            warm = psp.tile([P, N], f32, tag="ps")
            for t in range(NT):
                nc.tensor.transpose(
                    warm[:, t * P : (t + 1) * P], ident[:], ident[:]
                )

            # ---------------- phase A: build UT / VT / qsq for all slices ----
            # prefetch: all 8 input DMAs in flight up front
            lds = []
            for b in range(B):
                for (src, is_k) in ((Qd, False), (Kd, True)):
                    ld = ldp.tile([P, NT, R], f32, tag="ld")
                    if not is_k:
                        nc.gpsimd.memset(ld[:], -0.5)
                    nc.sync.dma_start(
                        ld[:, :, 0:D], src[b].rearrange("(t p) d -> p t d", p=P)
                    )
                    lds.append(ld)
            UTs, VTs, qsqs = [], [], []
            copy_idx = [0]

            def emit_phase_a(b):
                for (src, is_k) in ((Qd, False), (Kd, True)):
                    ld = lds[b * 2 + (1 if is_k else 0)]
                    sqt = sqtp.tile([P, NT, D], f32, tag="sqt")
                    nc.vector.tensor_tensor(
                        sqt[:], ld[:, :, 0:D], ld[:, :, 0:D], Alu.mult
                    )
                    if is_k:
                        ksq = ksqp.tile([P, NT], f32, tag="ksq")
                        nc.vector.tensor_reduce(ksq[:], sqt[:], AX.X, Alu.add)
                        khil = ksqp.tile([P, NT], f16, tag="khil")
                        nc.vector.tensor_copy(khil[:], ksq[:])
                        nc.vector.tensor_copy(ld[:, :, D], khil[:])
                        nc.vector.tensor_tensor(
                            ld[:, :, D + 1], ksq[:], ld[:, :, D], Alu.subtract
                        )
                    else:
                        qsq = qsqp.tile([P, NT], f32, tag="qsq")
                        nc.vector.tensor_reduce(qsq[:], sqt[:], AX.X, Alu.add)
                        qsqs.append(qsq)
                    tp = psp.tile([R, N], f32, tag="ps")
                    for t in range(NT):
                        nc.tensor.transpose(
                            tp[:, t * P : (t + 1) * P], ld[:, t, :], ident[:]
                        )
                    TT = uvp.tile([R, N], f16, tag="uv")
                    if copy_idx[0] < ACT_COPIES:
                        nc.scalar.copy(TT[:], tp[:])
                    else:
                        nc.vector.tensor_copy(TT[:], tp[:])
                    copy_idx[0] += 1
                    (VTs if is_k else UTs).append(TT)

            emit_phase_a(0)

            # ---------------- phases B-D per slice ----------------
            def emit_phase_d_piece(b, sts, c0, c1, s, last=False):
                # One supertile's affine + store.  Distributed piecewise into
                # the NEXT slice's supertile loop so the in-order ACT/DVE
                # queues interleave affines between that slice's sqrt drains
                # (instead of parking a 4-burst behind them), the DMA stream
                # stays smooth, and sq bufs recycle early.
                # last slice: ALL affines on DVE -- the tail is DMA-bound and
                # the ACT queue lags the DVE queue there, so an ACT affine
                # would only delay the first tail DMA.  Earlier slices:
                # alternate ACT_AFFINES of the 12 onto ACT (ACT has ~55us of
                # slack vs the DVE bottleneck; 7.2us there buys 2.5us here).
                st = sts[s]
                g = b * ST + s
                if not last and (
                    (g % 2 == 0 and g // 2 < ACT_AFFINES) or g == EXTRA_ACT_AFF
                ):
                    nc.scalar.activation(
                        st[:],
                        st[:],
                        AF.Identity,
                        bias=c0[:, 0:1],
                        scale=c1[:, 0:1],
                    )
                elif last:
                    # tail pieces: affine+store in HALF-supertile chunks so
                    # the DMA-bound tail stream starts ~1.2us earlier
                    half = STW * N // 2
                    for h in range(2):
                        nc.vector.tensor_scalar(
                            st[:, h * half : (h + 1) * half],
                            st[:, h * half : (h + 1) * half],
                            c1[:, 0:1], c0[:, 0:1], Alu.mult, Alu.add,
                        )
                        nc.sync.dma_start(
                            Od[
                                b,
                                (s * STW + h * 2) * P : (s * STW + (h + 1) * 2) * P,
                                :,
                            ].rearrange("(t p) n -> p t n", p=P),
                            st[:, h * half : (h + 1) * half].rearrange(
                                "p (t n) -> p t n", n=N
                            ),
                        )
                    return
                else:
                    nc.vector.tensor_scalar(
                        st[:], st[:], c1[:, 0:1], c0[:, 0:1], Alu.mult, Alu.add
                    )
                nc.sync.dma_start(
                    Od[b, s * STW * P : (s + 1) * STW * P, :].rearrange(
                        "(t p) n -> p t n", p=P
                    ),
                    st[:].rearrange("p (t n) -> p t n", n=N),
                )

            pend = None
            for b in range(B):
                UT, VT, qsq = UTs[b], VTs[b], qsqs[b]
                minp = smallp.tile([P, ST], f32, tag="minp")
                maxp = smallp.tile([P, ST], f32, tag="maxp")
                sts = []
                for s in range(ST):
                    if s == 1 and b + 1 < B:
                        emit_phase_a(b + 1)
                    st = sqp.tile([P, STW * N], f16, tag="sq")
                    # min/max via fp16 TT fold trees (2x mode; accum ops are
                    # stuck at 1x for 16-bit so fold first, accum on 512).
                    # The first fold level is SPLIT into two q-tile pairs so
                    # folding starts after TWO sqrt drains instead of four --
                    # at slice starts and warmup the DVE otherwise idles
                    # ~4-6us waiting for the full supertile.
                    # minp holds NEGATED minima (max of -x at the accum).
                    tas = {}
                    for t in range(STW):
                        i = s * STW + t
                        d2 = psp.tile([P, N], f32, tag="ps")
                        lhs = UT[:, i * P : (i + 1) * P]
                        for j in range(N // MM_N):
                            nc.tensor.matmul(
                                d2[:, j * MM_N : (j + 1) * MM_N],
                                lhs,
                                VT[:, j * MM_N : (j + 1) * MM_N],
                                start=True,
                                stop=True,
                            )
                        nc.scalar.activation(
                            st[:, t * N : (t + 1) * N],
                            d2[:],
                            AF.Sqrt,
                            bias=qsq[:, i : i + 1],
                            scale=-2.0,
                        )
                        if t == 1:
                            for (alu, sub) in ((Alu.max, "M"), (Alu.min, "m")):
                                ta = scrp.tile([P, 2048], f16, tag=f"ta{sub}")
                                nc.vector.tensor_tensor(
                                    ta[:], st[:, 0:2048], st[:, 2048:4096], alu
                                )
                                tas[sub] = ta
                    for (alu, sgn, sub, partial) in (
                        (Alu.max, 1.0, "M", maxp),
                        (Alu.min, -1.0, "m", minp),
                    ):
                        ta = tas[sub]
                        tb = scrp.tile([P, 2048], f16, tag=f"tb{sub}")
                        nc.vector.tensor_tensor(
                            tb[:], st[:, 4096:6144], st[:, 6144:8192], alu
                        )
                        # combine pairs in place: ta = alu(ta, tb)
                        nc.vector.tensor_tensor(ta[:], ta[:], tb[:], alu)
                        t3 = scrp.tile([P, 1024], f16, tag="t3")
                        nc.vector.tensor_tensor(
                            t3[:], ta[:, 0:1024], ta[:, 1024:2048], alu
                        )
                        t4 = scrp.tile([P, 512], f16, tag="t4")
                        nc.vector.tensor_tensor(
                            t4[:], t3[:, 0:512], t3[:, 512:1024], alu
                        )
                        t5 = scrp.tile([P, 512], f16, tag="t5")
                        nc.vector.tensor_scalar(
                            t5[:],
                            t4[:],
                            sgn,
                            None,
                            Alu.mult,
                            Alu.max,
                            accum_out=partial[:, s : s + 1],
                        )
                    sts.append(st)
                    # prev slice's supertile-s affine+store, emitted AFTER
                    # this supertile's folds: the in-order ACT queue then
                    # never parks an affine between a supertile's sqrt
                    # drains and the DVE folds waiting on them
                    if pend is not None and s < ST - 1:
                        emit_phase_d_piece(*pend, s)

                # ---------------- phase C: finalize scalars ----------------
                s2 = smallp.tile([P, 2], f32, tag="s2")
                sr = smallp.tile([P, 2], f32, tag="sr")
                u = smallp.tile([P, 1], f32, tag="u")
                r = smallp.tile([P, 1], f32, tag="r")
                c1 = smallp.tile([P, 1], f32, tag="c1")
                t0 = smallp.tile([P, 1], f32, tag="t0")
                c0 = smallp.tile([P, 1], f32, tag="c0")

                nc.vector.tensor_reduce(s2[:, 0:1], minp[:], AX.X, Alu.max)
                nc.vector.tensor_reduce(s2[:, 1:2], maxp[:], AX.X, Alu.max)
                nc.gpsimd.partition_all_reduce(
                    sr[:], s2[:], P, bass_isa.ReduceOp.max
                )
                # prev slice's final supertile executes on DVE/ACT while
                # gpsimd runs the partition reduce for this slice
                if pend is not None:
                    emit_phase_d_piece(*pend, ST - 1)
                    pend = None
                nmn = sr[:, 0:1]
                mx = sr[:, 1:2]
                nc.vector.tensor_tensor(u[:], mx, nmn, Alu.add)
                nc.vector.reciprocal(r[:], u[:])
                nc.vector.tensor_scalar(c1[:], r[:], A_HI - A_LO, None, Alu.mult)
                nc.vector.tensor_tensor(t0[:], nmn, c1[:], Alu.mult)
                nc.vector.tensor_scalar(c0[:], t0[:], A_LO, None, Alu.add)

                pend = (b, sts, c0, c1)
            for s in range(ST):
                emit_phase_d_piece(*pend, s, last=True)

    nc.compile()
    return nc


def get_program():
    if "nc" not in _CACHE:
        _CACHE["nc"] = build_program()
    return _CACHE["nc"]


def run(inputs, trace=False):
    Q = np.ascontiguousarray(np.asarray(inputs["Q"], dtype=np.float32))
    K = np.ascontiguousarray(np.asarray(inputs["K"], dtype=np.float32))
    assert Q.shape == (H, B, N, D) and K.shape == (H, B, N, D)

    from concourse.bass_utils import run_bass_kernel_spmd

    nc = get_program()
    in_maps = [{"Q": Q[h], "K": K[h]} for h in range(H)]
    res = run_bass_kernel_spmd(nc, in_maps, core_ids=list(range(H)), trace=trace)
    out = np.stack(
        [np.asarray(res.results[h]["out"]) for h in range(H)], axis=0
    ).astype(np.float32)
    return out, res


def kernel(**inputs) -> np.ndarray:
    out, _ = run(inputs, trace=False)
    return out


if __name__ == "__main__":
    nc = get_program()
    print("build ok:", nc)
